# revision 48
# baseline (speedup 1.0000x reference)
"""Trainium2 Bass kernel for nn_CIC (curve-attention GNN message passing block).

Sharding: data-parallel over batch B=16 -> 2 batches per core x 8 cores.
All weights replicated; no collectives.

Math (per batch b):
  att[n,l]   = sum_c w_att[c] * curves[c,n,l]
  S_l        = softmax(att, axis=l);  S_n = softmax(att, axis=n)
  curver_inter[c,n] = sum_l curves[c,n,l] * S_l[n,l]
  curves_intra[c,l] = sum_n curves[c,n,l] * S_n[n,l]
  a = wa @ curver_inter              [MID, n]
  b = wb @ curves_intra              [MID, l]
  AiT = wc.T @ a   [C, n];  AtT = wc.T @ b  [C, l]      (folded x_logits)
  logits_i = AiT.T @ x  [n, Np];  logits_t = AtT.T @ x  [l, Np]
  E = exp(logits); den = colsum(E)  (ones-matmul, replicated rows)
  UiT = a.T @ (wd[:, :MID] @ wn).T * s + beta    [n, C]  (BN folded; beta
        rides UiT because softmax columns sum to one)
  UtT = b.T @ (wd[:, MID:] @ wl).T * s   [l, C]
  y[c,p] = x[c,p] + sum_n UiT[n,c] Ei_n[n,p] + sum_l UtT[l,c] Et_n[l,p]
  out = leaky_relu(y, 0.2)   (x folded into the PSUM accumulation via I@x;
        lrelu(v) = 0.2 v + 0.8 relu(v) on the DVE, since a DVE op may read
        PSUM through only one operand)

Structure: phase 1 runs BOTH batches' curve preprocessing, phase 2 runs
both points loops back to back.  The points loop is software-pipelined in
groups of G chunks: group g's front half (x load, logits, Exp,
denominator matmuls, f32->bf16 staging) is emitted before group g-1's
back half (batched ACT Reciprocals, normalize, output matmuls, lrelu,
store), so the ACT engine runs G Exps then G Reciprocals back to back
and the activation table swaps 2x per group instead of 2x per chunk.
Output is stored bf16 and widened on the host.
"""

import numpy as np
import ml_dtypes

import concourse.bass as bass
import concourse.mybir as mybir
from concourse.tile import TileContext
from concourse.bass_utils import run_bass_kernel_spmd

bf16 = ml_dtypes.bfloat16
F32 = mybir.dt.float32
BF = mybir.dt.bfloat16
F32R = mybir.dt.float32r
ALU = mybir.AluOpType
AF = mybir.ActivationFunctionType
AX = mybir.AxisListType

B, C, N = 16, 256, 8192
CN, CL, MID = 128, 64, 128
NCORES = 8
NB = B // NCORES          # batches per core
NT = CN // 2              # 64 nl-tiles of 128 per batch
CHW = 512                 # points chunk width
NCH = N // CHW            # 16 chunks
G = 8                     # chunks per pipeline group
BN_EPS = 1e-5


_WS_COUNTER = [0]


def _split_excess_waits(nc, max_waits=1):
    """This walrus build rejects instructions carrying more than ~1 sem-wait
    command.  Move excess waits onto same-engine NoOps inserted right before
    the offending instruction (program order on one engine preserves the
    semantics exactly)."""
    for fn in nc.m.functions:
        for blk in fn.blocks:
            insts = list(blk.instructions)
            out = []
            changed = False
            for inst in insts:
                si = inst.sync_info
                waits = list(si.on_wait) if si and si.on_wait else []
                if len(waits) > max_waits and inst.engine is not None:
                    keep = waits[:max_waits]
                    extra = waits[max_waits:]
                    for w in extra:
                        _WS_COUNTER[0] += 1
                        nop = mybir.InstNoOp(
                            name=f"I-waitsplit-{_WS_COUNTER[0]}",
                            opcode="NoOp",
                            engine=inst.engine,
                            ins=[],
                            outs=[],
                            sync_info=mybir.SyncInfo(on_wait=[w], on_update=[]),
                        )
                        out.append(nop)
                    si.on_wait = keep
                    changed = True
                out.append(inst)
            if changed:
                blk.instructions = out


def _r(ap):
    """View an fp32 AP as float32r for full-rate PE streaming."""
    return ap.bitcast(F32R)


def _act_recip(nc, out_ap, in_ap):
    """ACT-engine reciprocal via raw InstActivation (the bass wrapper refuses
    Reciprocal for accuracy reasons; softmax denominators tolerate it)."""
    eng = nc.scalar
    imm = lambda v: mybir.ImmediateValue(dtype=mybir.dt.float32, value=v)
    return eng.add_instruction(
        mybir.InstActivation(
            name=nc.get_next_instruction_name(),
            func=AF.Reciprocal,
            ins=[eng.lower_ap(in_ap), imm(0.0), imm(1.0), imm(0.0)],
            outs=[eng.lower_ap(out_ap)],
        )
    )


def _build_bass():
    nc = bass.Bass()

    x_d = nc.dram_tensor("x", [NB, 2, 128, N], F32R, kind="ExternalInput")
    cu_d = nc.dram_tensor("curves", [NB, C, N], F32, kind="ExternalInput")
    wc_d = nc.dram_tensor("wc", [MID, C], F32R, kind="ExternalInput")
    waT_d = nc.dram_tensor("waT", [C, MID], BF, kind="ExternalInput")
    wbT_d = nc.dram_tensor("wbT", [C, MID], BF, kind="ExternalInput")
    wdnT_d = nc.dram_tensor("wdnT", [MID, C], F32R, kind="ExternalInput")
    wdlT_d = nc.dram_tensor("wdlT", [MID, C], F32R, kind="ExternalInput")
    wattr_d = nc.dram_tensor("wattr", [128, C], BF, kind="ExternalInput")
    onesn_d = nc.dram_tensor("onesn", [128, 128], BF, kind="ExternalInput")
    onesl_d = nc.dram_tensor("onesl", [64, 64], BF, kind="ExternalInput")
    i2_d = nc.dram_tensor("i2", [128, 64], BF, kind="ExternalInput")
    eyef_d = nc.dram_tensor("eyef", [128, 128], F32, kind="ExternalInput")
    eyer_d = nc.dram_tensor("eyer", [128, 128], F32R, kind="ExternalInput")
    eyeb_d = nc.dram_tensor("eyeb", [128, 128], BF, kind="ExternalInput")
    betar_d = nc.dram_tensor("betar", [128, C], F32, kind="ExternalInput")
    y_d = nc.dram_tensor("y", [NB, 2, 128, N], BF, kind="ExternalOutput")

    from contextlib import ExitStack

    with TileContext(nc) as tc:
        with ExitStack() as es0:
            # ---- long-lived pools ----
            cpool = es0.enter_context(tc.tile_pool(name="const", bufs=1))
            apool = es0.enter_context(tc.tile_pool(name="abuf", bufs=2))
            xpool = es0.enter_context(tc.tile_pool(name="xin", bufs=9))

            # ---- constants ----
            wc_t = cpool.tile([MID, C], F32R, tag="wc")
            nc.sync.dma_start(wc_t[:], wc_d[:])
            waT_c = []
            wbT_c = []
            for h in range(2):
                wat = cpool.tile([128, MID], BF, tag=f"waT{h}", name=f"waT{h}")
                nc.sync.dma_start(wat[:], waT_d[128 * h : 128 * (h + 1), :])
                waT_c.append(wat)
                wbt = cpool.tile([128, MID], BF, tag=f"wbT{h}", name=f"wbT{h}")
                nc.sync.dma_start(wbt[:], wbT_d[128 * h : 128 * (h + 1), :])
                wbT_c.append(wbt)
            wdnT_t = cpool.tile([MID, C], F32R, tag="wdnT")
            nc.sync.dma_start(wdnT_t[:], wdnT_d[:])
            wdlT_t = cpool.tile([MID, C], F32R, tag="wdlT")
            nc.sync.dma_start(wdlT_t[:], wdlT_d[:])
            wattr_t = cpool.tile([128, C], BF, tag="wattr")
            nc.sync.dma_start(wattr_t[:], wattr_d[:])
            onesn_t = cpool.tile([128, 128], BF, tag="onesn")
            nc.sync.dma_start(onesn_t[:], onesn_d[:])
            onesl_t = cpool.tile([64, 64], BF, tag="onesl")
            nc.sync.dma_start(onesl_t[:], onesl_d[:])
            i2_t = cpool.tile([128, 64], BF, tag="i2")
            nc.sync.dma_start(i2_t[:], i2_d[:])
            eyef_t = cpool.tile([128, 128], F32, tag="eyef")
            nc.sync.dma_start(eyef_t[:], eyef_d[:])
            eyer_t = cpool.tile([128, 128], F32R, tag="eyer")
            nc.sync.dma_start(eyer_t[:], eyer_d[:])
            eyeb_t = cpool.tile([128, 128], BF, tag="eyeb")
            nc.sync.dma_start(eyeb_t[:], eyeb_d[:])
            betar_t = cpool.tile([128, C], F32, tag="betar")
            nc.sync.dma_start(betar_t[:], betar_d[:])

            AiT_all, AtT_all, UiT_all, UtT_all = [], [], [], []

            # ================= phase 1: preprocessing (both batches) ======
            with ExitStack() as es1:
                cnpool = es1.enter_context(tc.tile_pool(name="cnat", bufs=6))
                ctpool = es1.enter_context(tc.tile_pool(name="ctrans", bufs=2))
                prpool = es1.enter_context(tc.tile_pool(name="prod", bufs=2))
                zppool = es1.enter_context(tc.tile_pool(name="zp", bufs=1))
                dpool = es1.enter_context(tc.tile_pool(name="dstk", bufs=1))
                spool = es1.enter_context(tc.tile_pool(name="small", bufs=2))
                ptp = es1.enter_context(
                    tc.tile_pool(name="pt", bufs=1, space="PSUM")
                )
                pttp = es1.enter_context(
                    tc.tile_pool(name="ptt", bufs=5, space="PSUM")
                )
                pap = es1.enter_context(
                    tc.tile_pool(name="pacc", bufs=1, space="PSUM")
                )
                psp = es1.enter_context(
                    tc.tile_pool(name="psmall", bufs=1, space="PSUM")
                )

                # zp: block-diag S_l stationary, zeroed once (per-batch
                # copies overwrite only the live slots)
                zp = zppool.tile([128, NT, 130], BF, tag="zp")
                nc.gpsimd.memset(zp[:], 0.0)
                zpf = zp[:].rearrange("p t u -> p (t u)")

                # Curve load + PE transposes + att, pipelined in quarters
                # and INTERLEAVED across both batches so the PE transposes
                # stay dense while the other batch's att runs on the DVE.
                NQ = 4
                TQ = NT // NQ          # nl-tiles per quarter
                WQ = N // NQ           # curve columns per quarter
                cTs = []
                attms = []
                for b in range(NB):
                    cT = ctpool.tile([128, NT, 256], BF, tag="ctrans")
                    attm = spool.tile([128, NT], F32, tag="attm")
                    cTs.append(cT)
                    attms.append(attm)

                def frontend_quarter(b, q):
                    cT = cTs[b]
                    attm = attms[b]
                    cns = []
                    for cc in range(2):
                        cn = cnpool.tile([128, WQ], BF, tag="cnat",
                                         name="cn")
                        nc.gpsimd.dma_start(
                            cn[:],
                            cu_d[b, 128 * cc : 128 * (cc + 1),
                                 WQ * q : WQ * (q + 1)],
                        )
                        cns.append(cn)
                    for cc in range(2):
                        for t4 in range(TQ // 4):
                            pst = pttp.tile([128, 4, 128], BF, tag="pst")
                            for i in range(4):
                                tloc = 4 * t4 + i
                                nc.tensor.transpose(
                                    pst[:, i, :],
                                    cns[cc][:, 128 * tloc : 128 * (tloc + 1)],
                                    eyeb_t[:],
                                )
                            t0 = TQ * q + 4 * t4
                            dst = cT[:, t0 : t0 + 4,
                                     128 * cc : 128 * (cc + 1)]
                            if (t4 + cc) % 4 == 0:
                                nc.vector.tensor_copy(dst, pst[:])
                            else:
                                nc.scalar.copy(dst, pst[:])
                    for sq in range(TQ // 4):
                        t0 = TQ * q + 4 * sq
                        prod = prpool.tile([128, 4, 256], BF, tag="prod")
                        wattr_b = wattr_t[:].rearrange(
                            "p (o c) -> p o c", o=1
                        ).broadcast_to([128, 4, 256])
                        nc.vector.tensor_tensor(
                            prod[:], cT[:, t0 : t0 + 4, :],
                            wattr_b, op=ALU.mult,
                        )
                        nc.vector.reduce_sum(
                            attm[:, t0 : t0 + 4], prod[:], axis=AX.X
                        )

                for q in range(NQ):
                    for b in range(NB):
                        frontend_quarter(b, q)
                fronts = []
                for b in range(NB):
                    # attm[q, t] = att[nl = 128 t + q]
                    amT_ps = ptp.tile([NT, 128], F32, tag="tps")
                    nc.tensor.transpose(amT_ps[:], attms[b][:], eyef_t[:])
                    amT = spool.tile([NT, 128], F32, tag="amT")
                    nc.scalar.copy(amT[:], amT_ps[:])
                    fronts.append((cTs[b], attms[b], amT))
                for b in range(NB):
                    cT, attm, amT = fronts[b]
                    # ---- softmax over l (rows of att) ----
                    rmax = spool.tile([NT, 2], F32, tag="rmax")
                    nc.vector.reduce_max(
                        rmax[:], amT[:].rearrange("t (j l) -> t j l", j=2),
                        axis=AX.X,
                    )
                    nmax = spool.tile([NT, 2], F32, tag="nmax")
                    nc.vector.tensor_scalar_mul(nmax[:], rmax[:], -1.0)
                    el = spool.tile([NT, 128], F32, tag="el")
                    for j in range(2):
                        nc.scalar.activation(
                            el[:, 64 * j : 64 * (j + 1)],
                            amT[:, 64 * j : 64 * (j + 1)],
                            AF.Exp,
                            bias=nmax[:, j : j + 1],
                            scale=1.0,
                        )
                    ssum = spool.tile([NT, 2], F32, tag="ssum")
                    nc.vector.reduce_sum(
                        ssum[:], el[:].rearrange("t (j l) -> t j l", j=2),
                        axis=AX.X,
                    )
                    rsum = spool.tile([NT, 2], F32, tag="rsum")
                    nc.vector.reciprocal(rsum[:], ssum[:])
                    slm = spool.tile([NT, 128], BF, tag="slm")
                    for j in range(2):
                        nc.vector.tensor_scalar_mul(
                            slm[:, 64 * j : 64 * (j + 1)],
                            el[:, 64 * j : 64 * (j + 1)],
                            rsum[:, j : j + 1],
                        )
                    slT_ps = ptp.tile([128, NT], BF, tag="tps")
                    nc.tensor.transpose(slT_ps[:], slm[:], eyeb_t[0:NT, 0:NT])
                    slT = spool.tile([128, NT], BF, tag="slT")
                    nc.vector.tensor_copy(slT[:], slT_ps[:])

                    # ---- softmax over n ----
                    m1 = spool.tile([128, 1], F32, tag="m1")
                    nc.vector.reduce_max(m1[:], attm[:], axis=AX.X)
                    m1u = spool.tile([64, 1], F32, tag="m1u")
                    nc.vector.tensor_copy(m1u[:], m1[64:128, :])
                    mc = spool.tile([64, 1], F32, tag="mc")
                    nc.vector.tensor_tensor(
                        mc[:], m1[0:64, :], m1u[:], op=ALU.max
                    )
                    nmc = spool.tile([64, 1], F32, tag="nmc")
                    nc.vector.tensor_scalar_mul(nmc[:], mc[:], -1.0)
                    nmf = spool.tile([128, 1], F32, tag="nmf")
                    nc.vector.tensor_copy(nmf[0:64, :], nmc[:])
                    nc.vector.tensor_copy(nmf[64:128, :], nmc[:])
                    en = spool.tile([128, NT], F32, tag="en")
                    nc.scalar.activation(
                        en[:], attm[:], AF.Exp, bias=nmf[:], scale=1.0
                    )
                    s1 = spool.tile([128, 1], F32, tag="s1")
                    nc.vector.reduce_sum(s1[:], en[:], axis=AX.X)
                    s1u = spool.tile([64, 1], F32, tag="s1u")
                    nc.vector.tensor_copy(s1u[:], s1[64:128, :])
                    sc = spool.tile([64, 1], F32, tag="sc")
                    nc.vector.tensor_tensor(
                        sc[:], s1[0:64, :], s1u[:], op=ALU.add
                    )
                    rc = spool.tile([64, 1], F32, tag="rc")
                    nc.vector.reciprocal(rc[:], sc[:])
                    rf = spool.tile([128, 1], F32, tag="rf")
                    nc.vector.tensor_copy(rf[0:64, :], rc[:])
                    nc.vector.tensor_copy(rf[64:128, :], rc[:])
                    snm = spool.tile([128, NT], F32, tag="snm")
                    nc.vector.tensor_scalar_mul(snm[:], en[:], rf[:])

                    # ---- write the 2 live columns of zp ----
                    nc.vector.tensor_copy(
                        zp[0:64, :, 0:1],
                        slT[0:64, :].rearrange("p (t o) -> p t o", o=1),
                    )
                    nc.vector.tensor_copy(
                        zp[64:128, :, 1:2],
                        slT[64:128, :].rearrange("p (t o) -> p t o", o=1),
                    )

                    # ---- curver_inter^T via 64 accumulated matmuls ----
                    ci_ps = pap.tile([128, 256], F32, tag="acc")
                    for t in range(NT):
                        nc.tensor.matmul(
                            ci_ps[:],
                            zpf[:, 128 * t : 128 * (t + 1)],
                            cT[:, t, :],
                            start=(t == 0),
                            stop=(t == NT - 1),
                        )
                    ciT = spool.tile([128, 256], BF, tag="ciT")
                    nc.scalar.copy(ciT[:], ci_ps[:])

                    # ---- all 64 dual-diag S_n stationaries in one op ----
                    dta = dpool.tile([128, NT, 64], BF, tag="dta")
                    nc.vector.tensor_tensor(
                        dta[:],
                        i2_t[:].rearrange("p (o l) -> p o l", o=1)
                        .broadcast_to([128, NT, 64]),
                        snm[:].rearrange("p (t o) -> p t o", o=1)
                        .broadcast_to([128, NT, 64]),
                        op=ALU.mult,
                    )

                    # ---- curves_intra^T via 64 accumulated matmuls ----
                    ct_ps = pap.tile([64, 256], F32, tag="acc")
                    for t in range(NT):
                        nc.tensor.matmul(
                            ct_ps[:],
                            dta[:, t, :],
                            cT[:, t, :],
                            start=(t == 0),
                            stop=(t == NT - 1),
                        )
                    ctT = spool.tile([64, 256], BF, tag="ctT")
                    nc.scalar.copy(ctT[:], ct_ps[:])

                    # ---- a = wa @ curver_inter ----
                    a_ps = psp.tile([128, 128], F32, tag="sm")
                    for h in range(2):
                        tr_ps = ptp.tile([128, 128], BF, tag="tps")
                        nc.tensor.transpose(
                            tr_ps[:], ciT[:, 128 * h : 128 * (h + 1)],
                            eyeb_t[:],
                        )
                        cin = spool.tile([128, 128], BF, tag="cin")
                        nc.vector.tensor_copy(cin[:], tr_ps[:])
                        nc.tensor.matmul(
                            a_ps[:], waT_c[h][:], cin[:],
                            start=(h == 0), stop=(h == 1),
                        )
                    a_sb = apool.tile([MID, 128], F32R, tag="a_sb")
                    nc.vector.tensor_copy(a_sb[:], a_ps[:])

                    # ---- b = wb @ curves_intra ----
                    b_ps = psp.tile([128, 64], F32, tag="sm")
                    for h in range(2):
                        tr2_ps = ptp.tile([128, 64], BF, tag="tps")
                        nc.tensor.transpose(
                            tr2_ps[:], ctT[:, 128 * h : 128 * (h + 1)],
                            eyeb_t[0:64, 0:64],
                        )
                        ctn = spool.tile([128, 64], BF, tag="ctn")
                        nc.vector.tensor_copy(ctn[:], tr2_ps[:])
                        nc.tensor.matmul(
                            b_ps[:], wbT_c[h][:], ctn[:],
                            start=(h == 0), stop=(h == 1),
                        )
                    b_sb = apool.tile([MID, 64], F32R, tag="b_sb")
                    nc.vector.tensor_copy(b_sb[:], b_ps[:])

                    # ---- AiT / AtT (wc.T @ a, wc.T @ b) ----
                    AiT = []
                    AtT = []
                    for cc in range(2):
                        ai_ps = psp.tile([128, 128], F32, tag="sm")
                        nc.tensor.matmul(
                            ai_ps[:],
                            _r(wc_t[:, 128 * cc : 128 * (cc + 1)]),
                            _r(a_sb[:]),
                            start=True,
                            stop=True,
                        )
                        ai = apool.tile(
                            [128, 128], F32R, tag=f"ai{cc}", name=f"ai{cc}"
                        )
                        nc.vector.tensor_copy(ai[:], ai_ps[:])
                        AiT.append(ai)
                        at_ps = psp.tile([128, 64], F32, tag="sm")
                        nc.tensor.matmul(
                            at_ps[:],
                            _r(wc_t[:, 128 * cc : 128 * (cc + 1)]),
                            _r(b_sb[:]),
                            start=True,
                            stop=True,
                        )
                        at = apool.tile(
                            [128, 64], F32R, tag=f"at{cc}", name=f"at{cc}"
                        )
                        nc.vector.tensor_copy(at[:], at_ps[:])
                        AtT.append(at)

                    # ---- UiT / UtT (beta folded into UiT) ----
                    ui_ps = psp.tile([128, 256], F32, tag="sm")
                    nc.tensor.matmul(
                        ui_ps[:], _r(a_sb[:]), _r(wdnT_t[:]),
                        start=True, stop=True,
                    )
                    UiT = apool.tile([128, 256], BF, tag="UiT")
                    nc.vector.tensor_tensor(
                        UiT[:], ui_ps[:], betar_t[:], op=ALU.add
                    )
                    ut_ps = psp.tile([64, 256], F32, tag="sm")
                    nc.tensor.matmul(
                        ut_ps[:], _r(b_sb[:]), _r(wdlT_t[:]),
                        start=True, stop=True,
                    )
                    UtT = apool.tile([64, 256], BF, tag="UtT")
                    nc.vector.tensor_copy(UtT[:], ut_ps[:])

                    AiT_all.append(AiT)
                    AtT_all.append(AtT)
                    UiT_all.append(UiT)
                    UtT_all.append(UtT)

            # ================= phase 2: points loops (both batches) =======
            with ExitStack() as es2:
                epool = es2.enter_context(tc.tile_pool(name="ebuf", bufs=3))
                dbpool = es2.enter_context(
                    tc.tile_pool(name="dstage", bufs=2)
                )
                rpool = es2.enter_context(tc.tile_pool(name="recip", bufs=2))
                enpool = es2.enter_context(tc.tile_pool(name="enorm", bufs=2))
                upool = es2.enter_context(tc.tile_pool(name="ubuf", bufs=2))
                opool = es2.enter_context(tc.tile_pool(name="obuf", bufs=3))
                plp = es2.enter_context(
                    tc.tile_pool(name="plog", bufs=5, space="PSUM")
                )
                pyp = es2.enter_context(
                    tc.tile_pool(name="py", bufs=3, space="PSUM")
                )

                def front_half(b, g):
                    """Loads, logits, Exp, denominators, f32->bf16 staging.
                    Per-group tiles (ei/et/dsb/dtb) are written in chunk
                    slices so the back half can consume them with ONE fat
                    instruction each — a single Reciprocal per group tile
                    cannot be interleaved with Exps by the scheduler."""
                    AiT = AiT_all[b]
                    AtT = AtT_all[b]
                    st = {"b": b, "g": g, "xts": []}
                    st["ei"] = epool.tile([128, G, CHW], BF, tag="ei", name="eig")
                    st["et"] = epool.tile([64, G, CHW], BF, tag="et", name="etg")
                    st["dsb"] = dbpool.tile([128, G, CHW], BF, tag="dsb", name="dsbg")
                    st["dtb"] = dbpool.tile([64, G, CHW], BF, tag="dtb", name="dtbg")
                    for j in range(G):
                        ks = CHW * (G * g + j)
                        xt = xpool.tile([128, 2, CHW], F32R, tag="xin")
                        nc.sync.dma_start(
                            xt[:],
                            x_d[b, :, :, ks : ks + CHW].rearrange(
                                "c p w -> p c w"
                            ),
                        )
                        pi = plp.tile([128, CHW], F32, tag="pl")
                        pt = plp.tile([64, CHW], F32, tag="pl")
                        for cc in range(2):
                            nc.tensor.matmul(
                                pi[:], AiT[cc][:], xt[:, cc, :],
                                start=(cc == 0), stop=(cc == 1),
                            )
                        for cc in range(2):
                            nc.tensor.matmul(
                                pt[:], AtT[cc][:], xt[:, cc, :],
                                start=(cc == 0), stop=(cc == 1),
                            )
                        ei = st["ei"][:, j, :]
                        nc.scalar.activation(ei, pi[:], AF.Exp)
                        et = st["et"][:, j, :]
                        nc.scalar.activation(et, pt[:], AF.Exp)

                        di = plp.tile([128, CHW], F32, tag="pl")
                        nc.tensor.matmul(
                            di[:], onesn_t[:], ei, start=True, stop=True
                        )
                        dt_ = plp.tile([64, CHW], F32, tag="pl")
                        nc.tensor.matmul(
                            dt_[:], onesl_t[:], et, start=True, stop=True
                        )
                        nc.vector.tensor_copy(st["dsb"][:, j, :], di[:])
                        nc.vector.tensor_copy(st["dtb"][:, j, :], dt_[:])
                        st["xts"].append(xt)
                    return st

                def back_half(st):
                    """Batched reciprocals, normalize, output, lrelu, store."""
                    b = st["b"]
                    g = st["g"]
                    UiT = UiT_all[b]
                    UtT = UtT_all[b]
                    ri = rpool.tile([128, G, CHW], BF, tag="ri")
                    _act_recip(nc, ri[:], st["dsb"][:])
                    rt = rpool.tile([64, G, CHW], BF, tag="rt")
                    _act_recip(nc, rt[:], st["dtb"][:])
                    ein_g = enpool.tile([128, G, CHW], BF, tag="ein")
                    nc.vector.tensor_tensor(
                        ein_g[:], st["ei"][:], ri[:], op=ALU.mult
                    )
                    etn_g = enpool.tile([64, G, CHW], BF, tag="etn")
                    nc.gpsimd.tensor_tensor(
                        etn_g[:], st["et"][:], rt[:], op=ALU.mult
                    )
                    for j in range(G):
                        ks = CHW * (G * g + j)
                        ein = ein_g[:, j, :]
                        etn = etn_g[:, j, :]
                        osb = opool.tile([128, 2, CHW], BF, tag="osb")
                        for cc in range(2):
                            yps = pyp.tile([128, CHW], F32, tag="yps")
                            nc.tensor.matmul(
                                yps[:], eyer_t[:], st["xts"][j][:, cc, :],
                                start=True, stop=False,
                            )
                            nc.tensor.matmul(
                                yps[:],
                                UiT[:, 128 * cc : 128 * (cc + 1)],
                                ein,
                                start=False,
                                stop=False,
                            )
                            nc.tensor.matmul(
                                yps[:],
                                UtT[:, 128 * cc : 128 * (cc + 1)],
                                etn,
                                start=False,
                                stop=True,
                            )
                            u = upool.tile([128, CHW], BF, tag="u")
                            nc.vector.tensor_scalar(
                                u[:], yps[:], 0.0, 0.8,
                                op0=ALU.max, op1=ALU.mult,
                            )
                            nc.vector.scalar_tensor_tensor(
                                osb[:, cc, :], yps[:], 0.2, u[:],
                                op0=ALU.mult, op1=ALU.add,
                            )
                        nc.sync.dma_start(
                            y_d[b, :, :, ks : ks + CHW].rearrange(
                                "c p w -> p c w"
                            ),
                            osb[:],
                        )

                prev = None
                for b in range(NB):
                    for g in range(NCH // G):
                        st = front_half(b, g)
                        if prev is not None:
                            back_half(prev)
                        prev = st
                back_half(prev)
    _split_excess_waits(nc, max_waits=1)
    return nc


_CACHE = {}


def _get_bass():
    if "nc" not in _CACHE:
        _CACHE["nc"] = _build_bass()
    return _CACHE["nc"]


def kernel(x, curves, w_att, wa, wb, wc, wn, wl, wd,
           bn_gamma, bn_beta, bn_mean, bn_var):
    x = np.ascontiguousarray(np.asarray(x, dtype=np.float32))
    curves = np.ascontiguousarray(np.asarray(curves, dtype=np.float32))
    w_att = np.asarray(w_att, dtype=np.float32)
    wa = np.asarray(wa, dtype=np.float32)
    wb = np.asarray(wb, dtype=np.float32)
    wc = np.asarray(wc, dtype=np.float32)
    wn = np.asarray(wn, dtype=np.float32)
    wl = np.asarray(wl, dtype=np.float32)
    wd = np.asarray(wd, dtype=np.float32)
    bn_gamma = np.asarray(bn_gamma, dtype=np.float32)
    bn_beta = np.asarray(bn_beta, dtype=np.float32)
    bn_mean = np.asarray(bn_mean, dtype=np.float32)
    bn_var = np.asarray(bn_var, dtype=np.float32)

    s = bn_gamma / np.sqrt(bn_var + BN_EPS)
    betap = (bn_beta - bn_mean * s).astype(np.float32)
    wdnT = ((wd[:, :MID] @ wn).T * s[None, :]).astype(np.float32)
    wdlT = ((wd[:, MID:] @ wl).T * s[None, :]).astype(np.float32)

    consts = {
        "wc": np.ascontiguousarray(wc),
        "waT": np.ascontiguousarray(wa.T).astype(bf16),
        "wbT": np.ascontiguousarray(wb.T).astype(bf16),
        "wdnT": np.ascontiguousarray(wdnT),
        "wdlT": np.ascontiguousarray(wdlT),
        "wattr": np.ascontiguousarray(
            np.broadcast_to(w_att.reshape(1, C), (128, C))
        ).astype(bf16),
        "onesn": np.ones((128, 128), bf16),
        "onesl": np.ones((64, 64), bf16),
        "i2": np.concatenate([np.eye(64), np.eye(64)], axis=0).astype(bf16),
        "eyef": np.eye(128, dtype=np.float32),
        "eyer": np.eye(128, dtype=np.float32),
        "eyeb": np.eye(128).astype(bf16),
        "betar": np.ascontiguousarray(
            np.broadcast_to(betap.reshape(1, C), (128, C))
        ).astype(np.float32),
    }

    curves_flat = curves.reshape(B, C, CN * CL)
    x_shaped = x.reshape(B, 2, 128, N)
    in_maps = []
    for core in range(NCORES):
        b0 = core * NB
        m = dict(consts)
        m["x"] = np.ascontiguousarray(x_shaped[b0 : b0 + NB])
        m["curves"] = np.ascontiguousarray(curves_flat[b0 : b0 + NB])
        in_maps.append(m)

    nc = _get_bass()
    res = run_bass_kernel_spmd(nc, in_maps, core_ids=list(range(NCORES)))
    out = np.empty((B, C, N), np.float32)
    for core in range(NCORES):
        out[core * NB : (core + 1) * NB] = (
            res.results[core]["y"].reshape(NB, C, N).astype(np.float32)
        )
    return out


# revision 49
# speedup vs baseline: 1.1539x; 1.1539x over previous
"""Trainium2 Bass kernel for nn_CIC (curve-attention GNN message passing block).

Sharding: data-parallel over batch B=16 -> 2 batches per core x 8 cores.
All weights replicated; no collectives.

Math (per batch b):
  att[n,l]   = sum_c w_att[c] * curves[c,n,l]
  S_l        = softmax(att, axis=l);  S_n = softmax(att, axis=n)
  curver_inter[c,n] = sum_l curves[c,n,l] * S_l[n,l]
  curves_intra[c,l] = sum_n curves[c,n,l] * S_n[n,l]
  a = wa @ curver_inter              [MID, n]
  b = wb @ curves_intra              [MID, l]
  AiT = wc.T @ a   [C, n];  AtT = wc.T @ b  [C, l]      (folded x_logits)
  logits_i = AiT.T @ x  [n, Np];  logits_t = AtT.T @ x  [l, Np]
  E = exp(logits); den = colsum(E)  (ones-matmul, replicated rows)
  UiT = a.T @ (wd[:, :MID] @ wn).T * s + beta    [n, C]  (BN folded; beta
        rides UiT because softmax columns sum to one)
  UtT = b.T @ (wd[:, MID:] @ wl).T * s   [l, C]
  y[c,p] = x[c,p] + sum_n UiT[n,c] Ei_n[n,p] + sum_l UtT[l,c] Et_n[l,p]
  out = leaky_relu(y, 0.2)   (x folded into the PSUM accumulation via I@x;
        lrelu(v) = 0.2 v + 0.8 relu(v) on the DVE, since a DVE op may read
        PSUM through only one operand)

Structure: phase 1 runs BOTH batches' curve preprocessing, phase 2 runs
both points loops back to back.  The points loop is software-pipelined in
groups of G chunks: group g's front half (x load, logits, Exp,
denominator matmuls, f32->bf16 staging) is emitted before group g-1's
back half (batched ACT Reciprocals, normalize, output matmuls, lrelu,
store), so the ACT engine runs G Exps then G Reciprocals back to back
and the activation table swaps 2x per group instead of 2x per chunk.
Output is stored bf16 and widened on the host.
"""

import numpy as np
import ml_dtypes

import concourse.bass as bass
import concourse.mybir as mybir
from concourse.tile import TileContext
from concourse.bass_utils import run_bass_kernel_spmd

bf16 = ml_dtypes.bfloat16
F32 = mybir.dt.float32
BF = mybir.dt.bfloat16
F32R = mybir.dt.float32r
ALU = mybir.AluOpType
AF = mybir.ActivationFunctionType
AX = mybir.AxisListType

B, C, N = 16, 256, 8192
CN, CL, MID = 128, 64, 128
NCORES = 8
NB = B // NCORES          # batches per core
NT = CN // 2              # 64 nl-tiles of 128 per batch
CHW = 512                 # points chunk width
NCH = N // CHW            # 16 chunks
G = 4                     # chunks per pipeline group
BN_EPS = 1e-5


_WS_COUNTER = [0]


def _split_excess_waits(nc, max_waits=1):
    """This walrus build rejects instructions carrying more than ~1 sem-wait
    command.  Move excess waits onto same-engine NoOps inserted right before
    the offending instruction (program order on one engine preserves the
    semantics exactly)."""
    for fn in nc.m.functions:
        for blk in fn.blocks:
            insts = list(blk.instructions)
            out = []
            changed = False
            for inst in insts:
                si = inst.sync_info
                waits = list(si.on_wait) if si and si.on_wait else []
                if len(waits) > max_waits and inst.engine is not None:
                    keep = waits[:max_waits]
                    extra = waits[max_waits:]
                    for w in extra:
                        _WS_COUNTER[0] += 1
                        nop = mybir.InstNoOp(
                            name=f"I-waitsplit-{_WS_COUNTER[0]}",
                            opcode="NoOp",
                            engine=inst.engine,
                            ins=[],
                            outs=[],
                            sync_info=mybir.SyncInfo(on_wait=[w], on_update=[]),
                        )
                        out.append(nop)
                    si.on_wait = keep
                    changed = True
                out.append(inst)
            if changed:
                blk.instructions = out


def _r(ap):
    """View an fp32 AP as float32r for full-rate PE streaming."""
    return ap.bitcast(F32R)


def _act_recip(nc, out_ap, in_ap):
    """ACT-engine reciprocal via raw InstActivation (the bass wrapper refuses
    Reciprocal for accuracy reasons; softmax denominators tolerate it)."""
    eng = nc.scalar
    imm = lambda v: mybir.ImmediateValue(dtype=mybir.dt.float32, value=v)
    return eng.add_instruction(
        mybir.InstActivation(
            name=nc.get_next_instruction_name(),
            func=AF.Reciprocal,
            ins=[eng.lower_ap(in_ap), imm(0.0), imm(1.0), imm(0.0)],
            outs=[eng.lower_ap(out_ap)],
        )
    )


def _build_bass():
    nc = bass.Bass()

    x_d = nc.dram_tensor("x", [NB, 2, 128, N], F32R, kind="ExternalInput")
    cu_d = nc.dram_tensor("curves", [NB, C, N], F32, kind="ExternalInput")
    wc_d = nc.dram_tensor("wc", [MID, C], F32R, kind="ExternalInput")
    waT_d = nc.dram_tensor("waT", [C, MID], BF, kind="ExternalInput")
    wbT_d = nc.dram_tensor("wbT", [C, MID], BF, kind="ExternalInput")
    wdnT_d = nc.dram_tensor("wdnT", [MID, C], F32R, kind="ExternalInput")
    wdlT_d = nc.dram_tensor("wdlT", [MID, C], F32R, kind="ExternalInput")
    wattr_d = nc.dram_tensor("wattr", [128, C], BF, kind="ExternalInput")
    onesn_d = nc.dram_tensor("onesn", [128, 128], BF, kind="ExternalInput")
    onesl_d = nc.dram_tensor("onesl", [64, 64], BF, kind="ExternalInput")
    i2_d = nc.dram_tensor("i2", [128, 64], BF, kind="ExternalInput")
    eyef_d = nc.dram_tensor("eyef", [128, 128], F32, kind="ExternalInput")
    eyer_d = nc.dram_tensor("eyer", [128, 128], F32R, kind="ExternalInput")
    eyeb_d = nc.dram_tensor("eyeb", [128, 128], BF, kind="ExternalInput")
    betar_d = nc.dram_tensor("betar", [128, C], F32, kind="ExternalInput")
    y_d = nc.dram_tensor("y", [NB, 2, 128, N], BF, kind="ExternalOutput")

    from contextlib import ExitStack

    with TileContext(nc) as tc:
        with ExitStack() as es0:
            # ---- long-lived pools ----
            cpool = es0.enter_context(tc.tile_pool(name="const", bufs=1))
            apool = es0.enter_context(tc.tile_pool(name="abuf", bufs=2))
            xpool = es0.enter_context(tc.tile_pool(name="xin", bufs=8))

            # ---- constants ----
            wc_t = cpool.tile([MID, C], F32R, tag="wc")
            nc.sync.dma_start(wc_t[:], wc_d[:])
            waT_c = []
            wbT_c = []
            for h in range(2):
                wat = cpool.tile([128, MID], BF, tag=f"waT{h}", name=f"waT{h}")
                nc.sync.dma_start(wat[:], waT_d[128 * h : 128 * (h + 1), :])
                waT_c.append(wat)
                wbt = cpool.tile([128, MID], BF, tag=f"wbT{h}", name=f"wbT{h}")
                nc.sync.dma_start(wbt[:], wbT_d[128 * h : 128 * (h + 1), :])
                wbT_c.append(wbt)
            wdnT_t = cpool.tile([MID, C], F32R, tag="wdnT")
            nc.sync.dma_start(wdnT_t[:], wdnT_d[:])
            wdlT_t = cpool.tile([MID, C], F32R, tag="wdlT")
            nc.sync.dma_start(wdlT_t[:], wdlT_d[:])
            wattr_t = cpool.tile([128, C], BF, tag="wattr")
            nc.sync.dma_start(wattr_t[:], wattr_d[:])
            onesn_t = cpool.tile([128, 128], BF, tag="onesn")
            nc.sync.dma_start(onesn_t[:], onesn_d[:])
            onesl_t = cpool.tile([64, 64], BF, tag="onesl")
            nc.sync.dma_start(onesl_t[:], onesl_d[:])
            i2_t = cpool.tile([128, 64], BF, tag="i2")
            nc.sync.dma_start(i2_t[:], i2_d[:])
            eyef_t = cpool.tile([128, 128], F32, tag="eyef")
            nc.sync.dma_start(eyef_t[:], eyef_d[:])
            eyer_t = cpool.tile([128, 128], F32R, tag="eyer")
            nc.sync.dma_start(eyer_t[:], eyer_d[:])
            eyeb_t = cpool.tile([128, 128], BF, tag="eyeb")
            nc.sync.dma_start(eyeb_t[:], eyeb_d[:])
            betar_t = cpool.tile([128, C], F32, tag="betar")
            nc.sync.dma_start(betar_t[:], betar_d[:])

            AiT_all, AtT_all, UiT_all, UtT_all = [], [], [], []

            # ================= phase 1: preprocessing (both batches) ======
            with ExitStack() as es1:
                cnpool = es1.enter_context(tc.tile_pool(name="cnat", bufs=6))
                ctpool = es1.enter_context(tc.tile_pool(name="ctrans", bufs=2))
                prpool = es1.enter_context(tc.tile_pool(name="prod", bufs=2))
                zppool = es1.enter_context(tc.tile_pool(name="zp", bufs=1))
                dpool = es1.enter_context(tc.tile_pool(name="dstk", bufs=1))
                spool = es1.enter_context(tc.tile_pool(name="small", bufs=2))
                ptp = es1.enter_context(
                    tc.tile_pool(name="pt", bufs=1, space="PSUM")
                )
                pttp = es1.enter_context(
                    tc.tile_pool(name="ptt", bufs=5, space="PSUM")
                )
                pap = es1.enter_context(
                    tc.tile_pool(name="pacc", bufs=1, space="PSUM")
                )
                psp = es1.enter_context(
                    tc.tile_pool(name="psmall", bufs=1, space="PSUM")
                )

                # zp: block-diag S_l stationary, zeroed once (per-batch
                # copies overwrite only the live slots)
                zp = zppool.tile([128, NT, 130], BF, tag="zp")
                nc.gpsimd.memset(zp[:], 0.0)
                zpf = zp[:].rearrange("p t u -> p (t u)")

                # Curve load + PE transposes + att, pipelined in quarters
                # and INTERLEAVED across both batches so the PE transposes
                # stay dense while the other batch's att runs on the DVE.
                NQ = 4
                TQ = NT // NQ          # nl-tiles per quarter
                WQ = N // NQ           # curve columns per quarter
                cTs = []
                attms = []
                for b in range(NB):
                    cT = ctpool.tile([128, NT, 256], BF, tag="ctrans")
                    attm = spool.tile([128, NT], F32, tag="attm")
                    cTs.append(cT)
                    attms.append(attm)

                def frontend_quarter(b, q):
                    cT = cTs[b]
                    attm = attms[b]
                    cns = []
                    for cc in range(2):
                        cn = cnpool.tile([128, WQ], BF, tag="cnat",
                                         name="cn")
                        nc.gpsimd.dma_start(
                            cn[:],
                            cu_d[b, 128 * cc : 128 * (cc + 1),
                                 WQ * q : WQ * (q + 1)],
                        )
                        cns.append(cn)
                    for cc in range(2):
                        for t4 in range(TQ // 4):
                            pst = pttp.tile([128, 4, 128], BF, tag="pst")
                            for i in range(4):
                                tloc = 4 * t4 + i
                                nc.tensor.transpose(
                                    pst[:, i, :],
                                    cns[cc][:, 128 * tloc : 128 * (tloc + 1)],
                                    eyeb_t[:],
                                )
                            t0 = TQ * q + 4 * t4
                            dst = cT[:, t0 : t0 + 4,
                                     128 * cc : 128 * (cc + 1)]
                            if (t4 + cc) % 4 == 0:
                                nc.vector.tensor_copy(dst, pst[:])
                            else:
                                nc.scalar.copy(dst, pst[:])
                    for sq in range(TQ // 4):
                        t0 = TQ * q + 4 * sq
                        prod = prpool.tile([128, 4, 256], BF, tag="prod")
                        wattr_b = wattr_t[:].rearrange(
                            "p (o c) -> p o c", o=1
                        ).broadcast_to([128, 4, 256])
                        nc.vector.tensor_tensor(
                            prod[:], cT[:, t0 : t0 + 4, :],
                            wattr_b, op=ALU.mult,
                        )
                        nc.vector.reduce_sum(
                            attm[:, t0 : t0 + 4], prod[:], axis=AX.X
                        )

                for q in range(NQ):
                    for b in range(NB):
                        frontend_quarter(b, q)
                fronts = []
                for b in range(NB):
                    # attm[q, t] = att[nl = 128 t + q]
                    amT_ps = ptp.tile([NT, 128], F32, tag="tps")
                    nc.tensor.transpose(amT_ps[:], attms[b][:], eyef_t[:])
                    amT = spool.tile([NT, 128], F32, tag="amT")
                    nc.scalar.copy(amT[:], amT_ps[:])
                    fronts.append((cTs[b], attms[b], amT))
                for b in range(NB):
                    cT, attm, amT = fronts[b]
                    # ---- softmax over l (rows of att) ----
                    rmax = spool.tile([NT, 2], F32, tag="rmax")
                    nc.vector.reduce_max(
                        rmax[:], amT[:].rearrange("t (j l) -> t j l", j=2),
                        axis=AX.X,
                    )
                    nmax = spool.tile([NT, 2], F32, tag="nmax")
                    nc.vector.tensor_scalar_mul(nmax[:], rmax[:], -1.0)
                    el = spool.tile([NT, 128], F32, tag="el")
                    for j in range(2):
                        nc.scalar.activation(
                            el[:, 64 * j : 64 * (j + 1)],
                            amT[:, 64 * j : 64 * (j + 1)],
                            AF.Exp,
                            bias=nmax[:, j : j + 1],
                            scale=1.0,
                        )
                    ssum = spool.tile([NT, 2], F32, tag="ssum")
                    nc.vector.reduce_sum(
                        ssum[:], el[:].rearrange("t (j l) -> t j l", j=2),
                        axis=AX.X,
                    )
                    rsum = spool.tile([NT, 2], F32, tag="rsum")
                    nc.vector.reciprocal(rsum[:], ssum[:])
                    slm = spool.tile([NT, 128], BF, tag="slm")
                    for j in range(2):
                        nc.vector.tensor_scalar_mul(
                            slm[:, 64 * j : 64 * (j + 1)],
                            el[:, 64 * j : 64 * (j + 1)],
                            rsum[:, j : j + 1],
                        )
                    slT_ps = ptp.tile([128, NT], BF, tag="tps")
                    nc.tensor.transpose(slT_ps[:], slm[:], eyeb_t[0:NT, 0:NT])
                    slT = spool.tile([128, NT], BF, tag="slT")
                    nc.vector.tensor_copy(slT[:], slT_ps[:])

                    # ---- softmax over n ----
                    m1 = spool.tile([128, 1], F32, tag="m1")
                    nc.vector.reduce_max(m1[:], attm[:], axis=AX.X)
                    m1u = spool.tile([64, 1], F32, tag="m1u")
                    nc.vector.tensor_copy(m1u[:], m1[64:128, :])
                    mc = spool.tile([64, 1], F32, tag="mc")
                    nc.vector.tensor_tensor(
                        mc[:], m1[0:64, :], m1u[:], op=ALU.max
                    )
                    nmc = spool.tile([64, 1], F32, tag="nmc")
                    nc.vector.tensor_scalar_mul(nmc[:], mc[:], -1.0)
                    nmf = spool.tile([128, 1], F32, tag="nmf")
                    nc.vector.tensor_copy(nmf[0:64, :], nmc[:])
                    nc.vector.tensor_copy(nmf[64:128, :], nmc[:])
                    en = spool.tile([128, NT], F32, tag="en")
                    nc.scalar.activation(
                        en[:], attm[:], AF.Exp, bias=nmf[:], scale=1.0
                    )
                    s1 = spool.tile([128, 1], F32, tag="s1")
                    nc.vector.reduce_sum(s1[:], en[:], axis=AX.X)
                    s1u = spool.tile([64, 1], F32, tag="s1u")
                    nc.vector.tensor_copy(s1u[:], s1[64:128, :])
                    sc = spool.tile([64, 1], F32, tag="sc")
                    nc.vector.tensor_tensor(
                        sc[:], s1[0:64, :], s1u[:], op=ALU.add
                    )
                    rc = spool.tile([64, 1], F32, tag="rc")
                    nc.vector.reciprocal(rc[:], sc[:])
                    rf = spool.tile([128, 1], F32, tag="rf")
                    nc.vector.tensor_copy(rf[0:64, :], rc[:])
                    nc.vector.tensor_copy(rf[64:128, :], rc[:])
                    snm = spool.tile([128, NT], F32, tag="snm")
                    nc.vector.tensor_scalar_mul(snm[:], en[:], rf[:])

                    # ---- write the 2 live columns of zp ----
                    nc.vector.tensor_copy(
                        zp[0:64, :, 0:1],
                        slT[0:64, :].rearrange("p (t o) -> p t o", o=1),
                    )
                    nc.vector.tensor_copy(
                        zp[64:128, :, 1:2],
                        slT[64:128, :].rearrange("p (t o) -> p t o", o=1),
                    )

                    # ---- curver_inter^T via 64 accumulated matmuls ----
                    ci_ps = pap.tile([128, 256], F32, tag="acc")
                    for t in range(NT):
                        nc.tensor.matmul(
                            ci_ps[:],
                            zpf[:, 128 * t : 128 * (t + 1)],
                            cT[:, t, :],
                            start=(t == 0),
                            stop=(t == NT - 1),
                        )
                    ciT = spool.tile([128, 256], BF, tag="ciT")
                    nc.scalar.copy(ciT[:], ci_ps[:])

                    # ---- all 64 dual-diag S_n stationaries in one op ----
                    dta = dpool.tile([128, NT, 64], BF, tag="dta")
                    nc.vector.tensor_tensor(
                        dta[:],
                        i2_t[:].rearrange("p (o l) -> p o l", o=1)
                        .broadcast_to([128, NT, 64]),
                        snm[:].rearrange("p (t o) -> p t o", o=1)
                        .broadcast_to([128, NT, 64]),
                        op=ALU.mult,
                    )

                    # ---- curves_intra^T via 64 accumulated matmuls ----
                    ct_ps = pap.tile([64, 256], F32, tag="acc")
                    for t in range(NT):
                        nc.tensor.matmul(
                            ct_ps[:],
                            dta[:, t, :],
                            cT[:, t, :],
                            start=(t == 0),
                            stop=(t == NT - 1),
                        )
                    ctT = spool.tile([64, 256], BF, tag="ctT")
                    nc.scalar.copy(ctT[:], ct_ps[:])

                    # ---- a = wa @ curver_inter ----
                    a_ps = psp.tile([128, 128], F32, tag="sm")
                    for h in range(2):
                        tr_ps = ptp.tile([128, 128], BF, tag="tps")
                        nc.tensor.transpose(
                            tr_ps[:], ciT[:, 128 * h : 128 * (h + 1)],
                            eyeb_t[:],
                        )
                        cin = spool.tile([128, 128], BF, tag="cin")
                        nc.vector.tensor_copy(cin[:], tr_ps[:])
                        nc.tensor.matmul(
                            a_ps[:], waT_c[h][:], cin[:],
                            start=(h == 0), stop=(h == 1),
                        )
                    a_sb = apool.tile([MID, 128], F32R, tag="a_sb")
                    nc.vector.tensor_copy(a_sb[:], a_ps[:])

                    # ---- b = wb @ curves_intra ----
                    b_ps = psp.tile([128, 64], F32, tag="sm")
                    for h in range(2):
                        tr2_ps = ptp.tile([128, 64], BF, tag="tps")
                        nc.tensor.transpose(
                            tr2_ps[:], ctT[:, 128 * h : 128 * (h + 1)],
                            eyeb_t[0:64, 0:64],
                        )
                        ctn = spool.tile([128, 64], BF, tag="ctn")
                        nc.vector.tensor_copy(ctn[:], tr2_ps[:])
                        nc.tensor.matmul(
                            b_ps[:], wbT_c[h][:], ctn[:],
                            start=(h == 0), stop=(h == 1),
                        )
                    b_sb = apool.tile([MID, 64], F32R, tag="b_sb")
                    nc.vector.tensor_copy(b_sb[:], b_ps[:])

                    # ---- AiT / AtT (wc.T @ a, wc.T @ b) ----
                    AiT = []
                    AtT = []
                    for cc in range(2):
                        ai_ps = psp.tile([128, 128], F32, tag="sm")
                        nc.tensor.matmul(
                            ai_ps[:],
                            _r(wc_t[:, 128 * cc : 128 * (cc + 1)]),
                            _r(a_sb[:]),
                            start=True,
                            stop=True,
                        )
                        ai = apool.tile(
                            [128, 128], F32R, tag=f"ai{cc}", name=f"ai{cc}"
                        )
                        nc.vector.tensor_copy(ai[:], ai_ps[:])
                        AiT.append(ai)
                        at_ps = psp.tile([128, 64], F32, tag="sm")
                        nc.tensor.matmul(
                            at_ps[:],
                            _r(wc_t[:, 128 * cc : 128 * (cc + 1)]),
                            _r(b_sb[:]),
                            start=True,
                            stop=True,
                        )
                        at = apool.tile(
                            [128, 64], F32R, tag=f"at{cc}", name=f"at{cc}"
                        )
                        nc.vector.tensor_copy(at[:], at_ps[:])
                        AtT.append(at)

                    # ---- UiT / UtT (beta folded into UiT) ----
                    ui_ps = psp.tile([128, 256], F32, tag="sm")
                    nc.tensor.matmul(
                        ui_ps[:], _r(a_sb[:]), _r(wdnT_t[:]),
                        start=True, stop=True,
                    )
                    UiT = apool.tile([128, 256], BF, tag="UiT")
                    nc.vector.tensor_tensor(
                        UiT[:], ui_ps[:], betar_t[:], op=ALU.add
                    )
                    ut_ps = psp.tile([64, 256], F32, tag="sm")
                    nc.tensor.matmul(
                        ut_ps[:], _r(b_sb[:]), _r(wdlT_t[:]),
                        start=True, stop=True,
                    )
                    UtT = apool.tile([64, 256], BF, tag="UtT")
                    nc.vector.tensor_copy(UtT[:], ut_ps[:])

                    AiT_all.append(AiT)
                    AtT_all.append(AtT)
                    UiT_all.append(UiT)
                    UtT_all.append(UtT)

            # ================= phase 2: points loops (both batches) =======
            with ExitStack() as es2:
                epool = es2.enter_context(tc.tile_pool(name="ebuf", bufs=3))
                dbpool = es2.enter_context(
                    tc.tile_pool(name="dstage", bufs=2)
                )
                rpool = es2.enter_context(tc.tile_pool(name="recip", bufs=2))
                enpool = es2.enter_context(tc.tile_pool(name="enorm", bufs=2))
                upool = es2.enter_context(tc.tile_pool(name="ubuf", bufs=2))
                opool = es2.enter_context(tc.tile_pool(name="obuf", bufs=3))
                plp = es2.enter_context(
                    tc.tile_pool(name="plog", bufs=5, space="PSUM")
                )
                pyp = es2.enter_context(
                    tc.tile_pool(name="py", bufs=3, space="PSUM")
                )

                def front_half(b, g):
                    """Loads, logits, Exp, denominators, f32->bf16 staging.
                    Per-group tiles (ei/et/dsb/dtb) are written in chunk
                    slices so the back half can consume them with ONE fat
                    instruction each — a single Reciprocal per group tile
                    cannot be interleaved with Exps by the scheduler."""
                    AiT = AiT_all[b]
                    AtT = AtT_all[b]
                    st = {"b": b, "g": g, "xts": []}
                    st["ei"] = epool.tile([128, G, CHW], BF, tag="ei", name="eig")
                    st["et"] = epool.tile([64, G, CHW], BF, tag="et", name="etg")
                    st["dsb"] = dbpool.tile([128, G, CHW], BF, tag="dsb", name="dsbg")
                    st["dtb"] = dbpool.tile([64, G, CHW], BF, tag="dtb", name="dtbg")
                    for j in range(G):
                        ks = CHW * (G * g + j)
                        xt = xpool.tile([128, 2, CHW], F32R, tag="xin")
                        nc.sync.dma_start(
                            xt[:],
                            x_d[b, :, :, ks : ks + CHW].rearrange(
                                "c p w -> p c w"
                            ),
                        )
                        pi = plp.tile([128, CHW], F32, tag="pl")
                        pt = plp.tile([64, CHW], F32, tag="pl")
                        for cc in range(2):
                            nc.tensor.matmul(
                                pi[:], AiT[cc][:], xt[:, cc, :],
                                start=(cc == 0), stop=(cc == 1),
                            )
                        for cc in range(2):
                            nc.tensor.matmul(
                                pt[:], AtT[cc][:], xt[:, cc, :],
                                start=(cc == 0), stop=(cc == 1),
                            )
                        ei = st["ei"][:, j, :]
                        nc.scalar.activation(ei, pi[:], AF.Exp)
                        et = st["et"][:, j, :]
                        nc.scalar.activation(et, pt[:], AF.Exp)

                        di = plp.tile([128, CHW], F32, tag="pl")
                        nc.tensor.matmul(
                            di[:], onesn_t[:], ei, start=True, stop=True
                        )
                        dt_ = plp.tile([64, CHW], F32, tag="pl")
                        nc.tensor.matmul(
                            dt_[:], onesl_t[:], et, start=True, stop=True
                        )
                        nc.vector.tensor_copy(st["dsb"][:, j, :], di[:])
                        nc.vector.tensor_copy(st["dtb"][:, j, :], dt_[:])
                        st["xts"].append(xt)
                    return st

                def back_half(st):
                    """Batched reciprocals, normalize, output, lrelu, store."""
                    b = st["b"]
                    g = st["g"]
                    UiT = UiT_all[b]
                    UtT = UtT_all[b]
                    ri = rpool.tile([128, G, CHW], BF, tag="ri")
                    _act_recip(nc, ri[:], st["dsb"][:])
                    rt = rpool.tile([64, G, CHW], BF, tag="rt")
                    _act_recip(nc, rt[:], st["dtb"][:])
                    ein_g = enpool.tile([128, G, CHW], BF, tag="ein")
                    nc.vector.tensor_tensor(
                        ein_g[:], st["ei"][:], ri[:], op=ALU.mult
                    )
                    etn_g = enpool.tile([64, G, CHW], BF, tag="etn")
                    nc.gpsimd.tensor_tensor(
                        etn_g[:], st["et"][:], rt[:], op=ALU.mult
                    )
                    for j in range(G):
                        ks = CHW * (G * g + j)
                        ein = ein_g[:, j, :]
                        etn = etn_g[:, j, :]
                        osb = opool.tile([128, 2, CHW], BF, tag="osb")
                        for cc in range(2):
                            yps = pyp.tile([128, CHW], F32, tag="yps")
                            nc.tensor.matmul(
                                yps[:], eyer_t[:], st["xts"][j][:, cc, :],
                                start=True, stop=False,
                            )
                            nc.tensor.matmul(
                                yps[:],
                                UiT[:, 128 * cc : 128 * (cc + 1)],
                                ein,
                                start=False,
                                stop=False,
                            )
                            nc.tensor.matmul(
                                yps[:],
                                UtT[:, 128 * cc : 128 * (cc + 1)],
                                etn,
                                start=False,
                                stop=True,
                            )
                            u = upool.tile([128, CHW], BF, tag="u")
                            nc.vector.tensor_scalar(
                                u[:], yps[:], 0.0, 0.8,
                                op0=ALU.max, op1=ALU.mult,
                            )
                            nc.vector.scalar_tensor_tensor(
                                osb[:, cc, :], yps[:], 0.2, u[:],
                                op0=ALU.mult, op1=ALU.add,
                            )
                        nc.sync.dma_start(
                            y_d[b, :, :, ks : ks + CHW].rearrange(
                                "c p w -> p c w"
                            ),
                            osb[:],
                        )

                prev = None
                for b in range(NB):
                    for g in range(NCH // G):
                        st = front_half(b, g)
                        if prev is not None:
                            back_half(prev)
                        prev = st
                back_half(prev)
    _split_excess_waits(nc, max_waits=1)
    return nc


_CACHE = {}


def _get_bass():
    if "nc" not in _CACHE:
        _CACHE["nc"] = _build_bass()
    return _CACHE["nc"]


def kernel(x, curves, w_att, wa, wb, wc, wn, wl, wd,
           bn_gamma, bn_beta, bn_mean, bn_var):
    x = np.ascontiguousarray(np.asarray(x, dtype=np.float32))
    curves = np.ascontiguousarray(np.asarray(curves, dtype=np.float32))
    w_att = np.asarray(w_att, dtype=np.float32)
    wa = np.asarray(wa, dtype=np.float32)
    wb = np.asarray(wb, dtype=np.float32)
    wc = np.asarray(wc, dtype=np.float32)
    wn = np.asarray(wn, dtype=np.float32)
    wl = np.asarray(wl, dtype=np.float32)
    wd = np.asarray(wd, dtype=np.float32)
    bn_gamma = np.asarray(bn_gamma, dtype=np.float32)
    bn_beta = np.asarray(bn_beta, dtype=np.float32)
    bn_mean = np.asarray(bn_mean, dtype=np.float32)
    bn_var = np.asarray(bn_var, dtype=np.float32)

    s = bn_gamma / np.sqrt(bn_var + BN_EPS)
    betap = (bn_beta - bn_mean * s).astype(np.float32)
    wdnT = ((wd[:, :MID] @ wn).T * s[None, :]).astype(np.float32)
    wdlT = ((wd[:, MID:] @ wl).T * s[None, :]).astype(np.float32)

    consts = {
        "wc": np.ascontiguousarray(wc),
        "waT": np.ascontiguousarray(wa.T).astype(bf16),
        "wbT": np.ascontiguousarray(wb.T).astype(bf16),
        "wdnT": np.ascontiguousarray(wdnT),
        "wdlT": np.ascontiguousarray(wdlT),
        "wattr": np.ascontiguousarray(
            np.broadcast_to(w_att.reshape(1, C), (128, C))
        ).astype(bf16),
        "onesn": np.ones((128, 128), bf16),
        "onesl": np.ones((64, 64), bf16),
        "i2": np.concatenate([np.eye(64), np.eye(64)], axis=0).astype(bf16),
        "eyef": np.eye(128, dtype=np.float32),
        "eyer": np.eye(128, dtype=np.float32),
        "eyeb": np.eye(128).astype(bf16),
        "betar": np.ascontiguousarray(
            np.broadcast_to(betap.reshape(1, C), (128, C))
        ).astype(np.float32),
    }

    curves_flat = curves.reshape(B, C, CN * CL)
    x_shaped = x.reshape(B, 2, 128, N)
    in_maps = []
    for core in range(NCORES):
        b0 = core * NB
        m = dict(consts)
        m["x"] = np.ascontiguousarray(x_shaped[b0 : b0 + NB])
        m["curves"] = np.ascontiguousarray(curves_flat[b0 : b0 + NB])
        in_maps.append(m)

    nc = _get_bass()
    res = run_bass_kernel_spmd(nc, in_maps, core_ids=list(range(NCORES)))
    out = np.empty((B, C, N), np.float32)
    for core in range(NCORES):
        out[core * NB : (core + 1) * NB] = (
            res.results[core]["y"].reshape(NB, C, N).astype(np.float32)
        )
    return out
